# revision 1
# baseline (speedup 1.0000x reference)
"""Bhattacharyya coefficient kernel for Trainium2 (8 NeuronCores, SPMD).

out[n,0,i,j] = (1/k^2) * sum_{c,p,q} w[c] * sqrt(x[n,c,i+p,j+q] * z[n,c,p,q])

Data-parallel over batch, 2 samples per core.  Per sample:
  1. ACT: sx = sqrt(x) (bf16), szw = w/k^2 * sqrt(z) (bf16); sqrt(x*z)
     factorizes so the whole unfold collapses into a cross-correlation.
  2. TensorE: plane[t, y] = sum_c szw[c, t] * sx[c, y] for the 64 taps
     t = 8p+q and all 63*63 pixels y (K=256 as two accumulating
     128-chunks, 512-column PSUM blocks).
  3. DVE evicts PSUM into fp8-e4m3 plane pieces sized exactly like the
     DRAM scratch tensors (one dump DMA each, exact dependencies).
     fp8 halves scratch traffic; the tap-sum averages 64 independent
     quantization errors so absmax rel err stays ~1.1e-2 (< 2e-2 gate).
  4. Dump to DRAM scratch (Sync ring: FIFO behind the x stream, i.e.
     the transfers fill the ramp-down of the stream) and gather back
     tap-aligned with a flat-DRAM diagonal AP
       A[t, u] = plane[t, u + 63*(t>>3) + (t&7)]     (SWDGE ring),
     which turns the tap-sum into a pure partition reduction.
  5. Strictly after both samples' stage-1 (engines run in near-emission
     order; interleaving that mismatches readiness serializes the
     kernel): per chunk a K=64 ones-matmul, column-tiled so chunk ch
     accumulates on PSUM partition 32ch.  Chunk 3 (which carries the
     last x block) gets its own PSUM and SBUF tiles so neither its
     matmul nor its eviction waits on tile-granular hazards against the
     chunk 0-2 path, and its eviction runs on the idle ACT engine.
     Chunks 0+1 and chunk 2 evict into separate obuf tiles so each out
     DMA's tile-granular RAW covers only its own eviction - rows 0-31
     ship as soon as chunks 0/1 reduce.

The x stream owns the Sync HWDGE ring (piece loads issued
back-to-back after the first piece + tiny z/w loads) and runs at the
~358 GB/s HBM limit for ~23us; everything else hides behind it except
sample 1's last-block tail.  Sample 1 ends with three small pieces
((4,2),(6,1),(7,1)) so its end-of-stream sqrt chain is short; its
gathers ride the drained Sync ring while sample 0 uses SWDGE, and
keep-warm matmuls bridge the post-stage-1 PE gap so HAM holds full
clock into the tail.  Measured 46.8-48.5us vs the 55.5us baseline;
run-to-run HW variance is +-3us.
"""

import numpy as np

import concourse.bacc as bacc
import concourse.bass as bass
import concourse.mybir as mybir
from concourse import tile
from concourse.bass_utils import run_bass_kernel_spmd

N, C, KS, MS = 16, 256, 8, 63
MO = MS - KS + 1            # 56
F = MS * MS                 # 3969
NCORES = 8
SPC = N // NCORES           # samples per core
BLK = 512
W = (MO - 1) * MS + MO      # 3521
SH = 448
AF = mybir.ActivationFunctionType
f32 = mybir.dt.float32
bf16 = mybir.dt.bfloat16
fp8 = mybir.dt.float8e4

PIECES = {0: [(0, 4), (4, 4)],
          1: [(0, 4), (4, 2), (6, 1), (7, 1)]}
GCH = [(0, 1008), (1008, 2016), (2016, 3024), (3024, W)]
SCR = [(0, 2016 + SH), (2016, 3024 + SH), (3024, F)]
CHUNK_SC = [0, 0, 1, 2]
OUT_ROWS = {1: (0, 32), 3: (32, MO)}

_CACHE = {}


def _build():
    nc = bacc.Bacc("TRN2", target_bir_lowering=False, debug=False)
    z_in = nc.declare_dram_parameter("z", [SPC, C, KS, KS], f32, isOutput=False)
    x_in = nc.declare_dram_parameter("x", [SPC, C, MS, MS], f32, isOutput=False)
    w_in = nc.declare_dram_parameter("w", [C], f32, isOutput=False)
    out = nc.declare_dram_parameter("out", [SPC, 1, MO, MO], f32, isOutput=True)

    scs = [
        [nc.dram_tensor(f"sc{ci}_{s}", [64, c1 - c0], fp8)
         for ci, (c0, c1) in enumerate(SCR)]
        for s in range(SPC)
    ]

    xflat = x_in.rearrange("s (k c) h w -> s k c (h w)", c=128)

    with tile.TileContext(nc) as tc:
        with (
            tc.tile_pool(name="xstage", bufs=12) as xstage,
            tc.tile_pool(name="sxq", bufs=5) as sxq,
            tc.tile_pool(name="zpool", bufs=2) as zpool,
            tc.tile_pool(name="plane", bufs=2) as plane,
            tc.tile_pool(name="gath", bufs=8) as gath,
            tc.tile_pool(name="opool", bufs=1) as opool,
            tc.tile_pool(name="psum", bufs=4, space="PSUM") as psum,
            tc.tile_pool(name="psum2", bufs=1, space="PSUM") as psum2,
            tc.tile_pool(name="psum3", bufs=2, space="PSUM") as psum3,
        ):
            xst = {}

            def load_piece(s, k, pi):
                b0, nbk = PIECES[s][pi]
                lo = b0 * BLK
                ln = min(nbk * BLK, F - lo)
                t = xstage.tile([128, 4 * BLK], f32, tag="xst",
                                name=f"xst{s}{k}{pi}")
                nc.sync.dma_start(t[:, :ln], xflat[s, k, :, lo : lo + ln])
                xst[(s, k, pi)] = t

            load_piece(0, 0, 0)
            wt = zpool.tile([128, 2], f32, name="wt")
            nc.sync.dma_start(wt[:], w_in.rearrange("(k c) -> c k", c=128))
            zts = []
            for s in range(SPC):
                zt = zpool.tile([128, 2, KS * KS], f32, tag="zt", name=f"zt{s}")
                nc.sync.dma_start(
                    zt[:], z_in[s].rearrange("(k c) p q -> c k (p q)", c=128)
                )
                zts.append(zt)
            for s in range(SPC):
                for pi in range(len(PIECES[s])):
                    for k in range(2):
                        if (s, k, pi) != (0, 0, 0):
                            load_piece(s, k, pi)

            ones = opool.tile([64, 1], fp8, name="ones")
            nc.gpsimd.memset(ones[:], 1.0)
            w64 = zpool.tile([128, 2], f32, name="w64")
            nc.vector.tensor_scalar_mul(w64[:], wt[:], 1.0 / (KS * KS))

            obufs, obufBs, obuf3s, psum2s, psum3s, ats = ({}, {}, {}, {},
                                                         {}, {})
            deferred_gathers = []
            for s in range(SPC):
                obuf = opool.tile([128, 1024], f32, tag=f"ob{s}",
                                  name=f"obuf{s}")
                obufs[s] = obuf
                obuf3s[s] = opool.tile([128, 512], f32, tag=f"ob3{s}",
                                       name=f"obuf3_{s}")
                obufBs[s] = opool.tile([128, 1024], f32, tag=f"obB{s}",
                                       name=f"obufB_{s}")
                psum2s[s] = psum2.tile([128, 2 * BLK], f32, tag="ps2",
                                       name=f"ps2_{s}")
                psum3s[s] = psum3.tile([128, BLK], f32, tag="ps3",
                                       name=f"ps3_{s}")
                zsq = zpool.tile([128, 2, KS * KS], f32, tag="zsq", name=f"zsq{s}")
                szw = zpool.tile([128, 2, KS * KS], bf16, tag="szw", name=f"szw{s}")
                for kk in range(2):
                    nc.scalar.activation(zsq[:, kk, :], zts[s][:, kk, :], AF.Sqrt)
                    nc.vector.tensor_scalar_mul(
                        szw[:, kk, :], zsq[:, kk, :], w64[:, kk : kk + 1]
                    )

                pls = [
                    plane.tile([64, c1 - c0], fp8, tag=f"pl{ci}",
                               name=f"pl{s}_{ci}")
                    for ci, (c0, c1) in enumerate(SCR)
                ]
                evmap = [[] for _ in range(8)]
                for ci, (c0, c1) in enumerate(SCR):
                    for b in range(8):
                        lo = max(c0, b * BLK)
                        hi = min(c1, (b + 1) * BLK, F)
                        if lo < hi:
                            evmap[b].append((ci, lo - b * BLK, hi - b * BLK,
                                             lo - c0))
                last_block = [min((c1 - 1) // BLK, 7) for (c0, c1) in SCR]

                def emit_stage2(ci):
                    c0, c1 = SCR[ci]
                    pit = c1 - c0
                    nc.sync.dma_start(scs[s][ci][:, :], pls[ci][:])
                    for ch, (u0, u1) in enumerate(GCH):
                        if CHUNK_SC[ch] != ci:
                            continue
                        ulen = u1 - u0
                        a = gath.tile([64, 1008], fp8, tag="a",
                                      name=f"a{s}_{ch}")
                        src = bass.AP(
                            scs[s][ci][:].tensor,
                            u0 - c0,
                            [[8 * pit + MS, 8], [pit + 1, 8], [1, ulen]],
                        )
                        if s == 0:
                            nc.gpsimd.dma_start(a[:, :ulen], src)
                            ats[(s, ch)] = a
                        else:
                            deferred_gathers.append((a, ulen, src))
                            ats[(s, ch)] = a

                for pi, (b0, nbk) in enumerate(PIECES[s]):
                    lo = b0 * BLK
                    ln = min(nbk * BLK, F - lo)
                    sxp = {}
                    for k in range(2):
                        t = sxq.tile([128, 4 * BLK], bf16, tag="sxp",
                                     name=f"sxp{s}{k}{pi}")
                        nc.scalar.activation(
                            t[:, :ln], xst[(s, k, pi)][:, :ln], AF.Sqrt
                        )
                        sxp[k] = t
                    for j in range(nbk):
                        b = b0 + j
                        nb = min(BLK, F - b * BLK)
                        ps = psum.tile([64, BLK], f32, tag="ps",
                                       name=f"ps_{s}_{b}")
                        for k in range(2):
                            nc.tensor.matmul(
                                ps[:, :nb],
                                szw[:, k, :],
                                sxp[k][:, j * BLK : j * BLK + nb],
                                start=(k == 0),
                                stop=(k == 1),
                            )
                        for (ci, p_lo, p_hi, d_lo) in evmap[b]:
                            nc.vector.tensor_copy(
                                pls[ci][:, d_lo : d_lo + (p_hi - p_lo)],
                                ps[:, p_lo:p_hi],
                            )
                        for ci in range(len(SCR)):
                            if last_block[ci] == b:
                                emit_stage2(ci)

            # sample 1's gathers ride the (drained) Sync ring, issued
            # after all three of its dumps so no gather's completion wait
            # head-blocks a later dump's issue
            for (a, ulen, src_ap) in deferred_gathers:
                nc.sync.dma_start(a[:, :ulen], src_ap)

            # keep-warm: PE idles ~2us here waiting for the first gathers;
            # >3us idle re-throttles HAM to half clock for every tail
            # matmul.  Dummy matmuls on resident data bridge the gap
            # (results never read; WAR on the psum pool spaces them).
            for wi in range(6):
                pd = psum.tile([64, BLK], f32, tag="ps", name=f"warm{wi}")
                nc.tensor.matmul(
                    pd[:, :385],
                    szw[:, 0, :],
                    sxp[0][:, 0:385],
                    start=True,
                    stop=True,
                )

            # ---- stage-2 compute, strictly after both samples' stage-1
            # so engine program order matches data readiness
            for s in range(SPC):
                ps2 = psum2s[s]

                def mm2(ch):
                    u0, u1 = GCH[ch]
                    ulen = u1 - u0
                    row = 32 * ch
                    dst = ps2 if ch < 3 else psum3s[s]
                    a = ats[(s, ch)]
                    for m0 in range(0, ulen, BLK):
                        nb = min(BLK, ulen - m0)
                        nc.tensor.matmul(
                            dst[row : row + 1, m0 : m0 + nb],
                            ones[:],
                            a[:, m0 : m0 + nb],
                            start=True,
                            stop=True,
                            tile_position=(0, row),
                        )

                ob = obufs[s]
                obB = obufBs[s]
                # chunks 0-2 -> rows 0-47 ship without waiting on chunk
                # 3's dump/gather chain (it carries the last x block)
                for ch in range(3):
                    mm2(ch)
                nc.vector.tensor_copy(ob[0:33, :], ps2[0:33, :])
                nc.vector.tensor_copy(obB[64:65, :], ps2[64:65, :])
                osrc = bass.AP(ob[:].tensor, 0,
                               [[32 * 1024, 2], [MS, 16], [1, MO]])
                nc.sync.dma_start(out[s, 0, 0:32].unsqueeze(0), osrc)
                osrc = obB[64:65, 0 : 16 * MS].rearrange(
                    "p (i j) -> p i j", i=16
                )[:, :, 0:MO]
                nc.sync.dma_start(out[s, 0, 32:48].unsqueeze(0), osrc)
                mm2(3)
                ob3 = obuf3s[s]
                nc.scalar.copy(ob3[96:97, 0:BLK],
                               psum3s[s][96:97, 0:BLK])
                osrc = ob3[96:97, 0 : 8 * MS].rearrange(
                    "p (i j) -> p i j", i=8
                )[:, :, 0:MO]
                nc.sync.dma_start(out[s, 0, 48:MO].unsqueeze(0), osrc)

    nc.compile()
    return nc


def _get_nc():
    if "nc" not in _CACHE:
        _CACHE["nc"] = _build()
    return _CACHE["nc"]


def _run(z, x, weights, **runkw):
    z = np.ascontiguousarray(np.asarray(z), dtype=np.float32)
    x = np.ascontiguousarray(np.asarray(x), dtype=np.float32)
    w = np.ascontiguousarray(np.asarray(weights), dtype=np.float32).reshape(C)
    in_maps = []
    for i in range(NCORES):
        lo, hi = i * SPC, (i + 1) * SPC
        in_maps.append({"z": z[lo:hi], "x": x[lo:hi], "w": w})
    nc = _get_nc()
    try:
        res = run_bass_kernel_spmd(
            nc, in_maps, core_ids=list(range(NCORES)), **runkw
        )
    except Exception:
        res = run_bass_kernel_spmd(
            nc, in_maps, core_ids=list(range(NCORES)), **runkw
        )
    full = np.concatenate([res.results[i]["out"] for i in range(NCORES)], axis=0)
    return full, res


def kernel(z, x, weights):
    full, _ = _run(z, x, weights)
    return full



# revision 23
# speedup vs baseline: 1.1486x; 1.1486x over previous
"""Bhattacharyya coefficient kernel for Trainium2 (8 NeuronCores, SPMD).

out[n,0,i,j] = (1/k^2) * sum_{c,p,q} w[c] * sqrt(x[n,c,i+p,j+q] * z[n,c,p,q])

Data-parallel over batch, 2 samples per core.  Per sample:
  1. ACT: sx = sqrt(x) (bf16), szw = w/k^2 * sqrt(z) (bf16); sqrt(x*z)
     factorizes so the whole unfold collapses into a cross-correlation.
  2. TensorE: plane[t, y] = sum_c szw[c, t] * sx[c, y] for the 64 taps
     t = 8p+q and all 63*63 pixels y (K=256 as two accumulating
     128-chunks, 512-column PSUM blocks).
  3. DVE evicts each PSUM block with one copy into fp8-e4m3 plane
     tiles in SBUF: "early" = cols 0-3583 (blocks 0-6; block 6 ends
     exactly at 3584) and "late" = cols 3584-3968 (block 7 only), so
     nothing the last x block produces gates the early scratch path.
     fp8 halves scratch traffic; the tap-sum averages 64 independent
     quantization errors so absmax rel err stays ~1.1e-2 (< 2e-2).
  4. Both tiles dump into one flat DRAM scratch [64, 3969] and gather
     back tap-aligned with flat-DRAM diagonal APs (SBUF APs reject
     mixed partition steps, so the bounce must go through DRAM):
       A[t, u] = plane[t, u + 63*(t>>3) + (t&7)]
     which turns the tap-sum into a pure partition reduction.  The
     tile split gives exact dump dependencies while the single flat
     tensor lets sample 0 use ONE 3521-wide gather and sample 1 a
     3136-wide gather (everything reachable from blocks 0-6) plus a
     385-wide tail gather -- no overlap duplication anywhere.
  5. Stage-2: K=64 ones-matmuls, column-tiled so each 1008-wide output
     group accumulates on PSUM partition 32g (pairs to different
     column groups run concurrently); ACT and DVE split the PSUM->SBUF
     result copies; small DMAs store the output rows.

Scheduling notes (all from HW traces):
  - z/w are re-laid-out on the host (pure reshape/broadcast) so their
    loads are >=512B-contiguous per partition; the natural layout
    emits hundreds of 4-256B descriptors that poison the SDMA queues
    and delay every later piece-completion semaphore by ~3us.
  - Everything rides the Sync HWDGE ring in data-flow order; the ring
    FIFO is the flow control.  All 13 x piece loads are issued before
    any scratch traffic so the stream never stalls.  SWDGE gets only
    ~25 GB/s of leftover fabric mid-stream, so it carries only sample
    0's post-stream output stores.
  - The tail after the last x block is the short chain sqrt -> matmul
    -> evict -> dump(385 cols) -> gather -> one matmul -> ACT evict ->
    store; sample 0's whole stage-2 and sample 1's rows 0-47 complete
    ahead of it on the draining ring.
  - A few dummy matmuls on resident data bridge the PE's idle gap
    before stage-2 so HAM keeps the array at full clock for the tail.
"""

import numpy as np

import concourse.bacc as bacc
import concourse.bass as bass
import concourse.mybir as mybir
from concourse import tile
from concourse.bass_utils import run_bass_kernel_spmd

N, C, KS, MS = 16, 256, 8, 63
MO = MS - KS + 1            # 56
F = MS * MS                 # 3969
NCORES = 8
SPC = N // NCORES           # samples per core
BLK = 512
W = (MO - 1) * MS + MO      # 3521
KK = KS * KS                # 64
EW = 7 * BLK                # early tile: cols 0-3583
LW = F - EW                 # late tile: 385 cols
AF = mybir.ActivationFunctionType
ALU = mybir.AluOpType
f32 = mybir.dt.float32
bf16 = mybir.dt.bfloat16
fp8 = mybir.dt.float8e4

PIECES = {0: [(0, 4), (4, 4)],
          1: [(0, 4), (4, 3), (7, 1)]}
OBW = 1024

_CACHE = {}


def _build():
    nc = bacc.Bacc("TRN2", target_bir_lowering=False, debug=False)
    z_in = nc.declare_dram_parameter("z", [SPC, 128, 2, KK], f32,
                                     isOutput=False)
    x_in = nc.declare_dram_parameter("x", [SPC, C, MS, MS], f32,
                                     isOutput=False)
    w_in = nc.declare_dram_parameter("w", [128, 2, KK], f32, isOutput=False)
    out = nc.declare_dram_parameter("out", [SPC, 1, MO, MO], f32, isOutput=True)

    scs = [nc.dram_tensor(f"sc_{s}", [64, F], fp8) for s in range(SPC)]

    xflat = x_in.rearrange("s (k c) h w -> s k c (h w)", c=128)

    with tile.TileContext(nc) as tc:
        with (
            tc.tile_pool(name="xstage", bufs=10) as xstage,
            tc.tile_pool(name="sxq", bufs=5) as sxq,
            tc.tile_pool(name="zpool", bufs=2) as zpool,
            tc.tile_pool(name="plane", bufs=2) as plane,
            tc.tile_pool(name="gath", bufs=2) as gath,
            tc.tile_pool(name="opool", bufs=1) as opool,
            tc.tile_pool(name="psum", bufs=4, space="PSUM") as psum,
            tc.tile_pool(name="psum2", bufs=1, space="PSUM") as psum2,
            tc.tile_pool(name="psum3", bufs=1, space="PSUM") as psum3,
        ):
            xst = {}

            def load_piece(s, k, pi):
                b0, nbk = PIECES[s][pi]
                lo = b0 * BLK
                ln = min(nbk * BLK, F - lo)
                t = xstage.tile([128, 4 * BLK], f32, tag="xst",
                                name=f"xst{s}{k}{pi}")
                nc.sync.dma_start(t[:, :ln], xflat[s, k, :, lo : lo + ln])
                xst[(s, k, pi)] = t

            # ---- the x stream: all 13 loads back-to-back on the Sync
            # ring; z/w (contiguous per-partition layout) near the front
            wb = zpool.tile([128, 2, KK], f32, name="wb")
            nc.sync.dma_start(wb[:], w_in[:])
            zts = []
            for s in range(SPC):
                zt = zpool.tile([128, 2, KK], f32, tag="zt", name=f"zt{s}")
                nc.sync.dma_start(zt[:], z_in[s])
                zts.append(zt)
            load_piece(0, 0, 0)
            load_piece(0, 1, 0)
            load_piece(0, 0, 1)
            load_piece(0, 1, 1)
            load_piece(1, 0, 0)
            load_piece(1, 1, 0)
            load_piece(1, 0, 1)
            load_piece(1, 1, 1)
            # (the last two small pieces are issued after sample 0's
            # dumps so the dumps transfer inside the stream window)

            # ---- template prep: szw = (sqrt(z) / k^2) * w
            ones = opool.tile([64, 1], fp8, name="ones")
            nc.gpsimd.memset(ones[:], 1.0)
            szws = []
            for s in range(SPC):
                zsq = zpool.tile([128, 2, KK], f32, tag="zsq", name=f"zsq{s}")
                szw = zpool.tile([128, 2, KK], bf16, tag="szw", name=f"szw{s}")
                nc.scalar.activation(zsq[:], zts[s][:], AF.Sqrt)
                nc.vector.scalar_tensor_tensor(
                    szw[:], zsq[:], 1.0 / KK, wb[:], ALU.mult, ALU.mult
                )
                szws.append(szw)

            def stage1(s):
                """sqrt + correlation matmuls + one fp8 eviction/block."""
                ple = plane.tile([64, EW], fp8, tag="ple", name=f"ple{s}")
                pll = plane.tile([64, LW], fp8, tag="pll", name=f"pll{s}")
                last_sxp = None
                for pi, (b0, nbk) in enumerate(PIECES[s]):
                    lo = b0 * BLK
                    ln = min(nbk * BLK, F - lo)
                    sxp = {}
                    for k in range(2):
                        t = sxq.tile([128, 4 * BLK], bf16, tag="sxp",
                                     name=f"sxp{s}{k}{pi}")
                        nc.scalar.activation(
                            t[:, :ln], xst[(s, k, pi)][:, :ln], AF.Sqrt
                        )
                        sxp[k] = t
                    last_sxp = sxp[0]
                    for j in range(nbk):
                        b = b0 + j
                        nb = min(BLK, F - b * BLK)
                        ps = psum.tile([64, BLK], f32, tag="ps",
                                       name=f"ps_{s}_{b}")
                        for k in range(2):
                            nc.tensor.matmul(
                                ps[:, :nb],
                                szws[s][:, k, :],
                                sxp[k][:, j * BLK : j * BLK + nb],
                                start=(k == 0),
                                stop=(k == 1),
                            )
                        if b < 7:
                            nc.vector.tensor_copy(
                                ple[:, b * BLK : b * BLK + nb], ps[:, :nb]
                            )
                        else:
                            nc.vector.tensor_copy(pll[:, :nb], ps[:, :nb])
                return ple, pll, last_sxp

            def gather_dma(s, u0, ulen, name):
                a = gath.tile([64, ulen], fp8, tag=name, name=f"{name}_{s}")
                src = bass.AP(
                    scs[s][:].tensor,
                    u0,
                    [[8 * F + MS, 8], [F + 1, 8], [1, ulen]],
                )
                nc.sync.dma_start(a[:], src)
                return a

            def mm2(a, au0, u0, u1, dst):
                # output group g (1008 wide) lands on PSUM partition 32g
                for g in range(u0 // 1008, (u1 + 1007) // 1008):
                    row = 32 * g
                    glo, ghi = max(u0, g * 1008), min(u1, (g + 1) * 1008)
                    for m0 in range(glo, ghi, BLK):
                        nb = min(BLK, ghi - m0)
                        nc.tensor.matmul(
                            dst[row : row + 1,
                                m0 - g * 1008 : m0 - g * 1008 + nb],
                            ones[:],
                            a[:, m0 - au0 : m0 - au0 + nb],
                            start=True,
                            stop=True,
                            tile_position=(0, row),
                        )

            # ---- stage-1 for both samples; sample 0's dumps slot into
            # the ring before the last two (small, late-needed) x loads
            # so their transfers ride inside the stream window
            ple0, pll0, _ = stage1(0)
            nc.sync.dma_start(scs[0][:, 0:EW], ple0[:])
            nc.sync.dma_start(scs[0][:, EW:F], pll0[:])
            load_piece(1, 0, 2)
            load_piece(1, 1, 2)
            ple1, pll1, wsxp = stage1(1)

            # keep-warm: bridge the PE idle gap before stage-2 so HAM
            # holds full clock (results never read; pool WAR spaces them)
            for wi in range(4):
                pd = psum.tile([64, BLK], f32, tag="ps", name=f"warm{wi}")
                nc.tensor.matmul(pd[:, :385], szws[1][:, 0, :],
                                 wsxp[:, 0:385], start=True, stop=True)

            # ---- s0 gather + stage-2 (ring-FIFO at stream end)
            aall_0 = gather_dma(0, 0, W, "aall")
            ps2_0 = psum2.tile([128, 2 * BLK], f32, tag="ps2", name="ps2_0")
            mm2(aall_0, 0, 0, W, ps2_0)
            ob0 = opool.tile([128, OBW], f32, tag="ob0", name="obuf0")
            nc.scalar.copy(ob0[0:97, 0:1008], ps2_0[0:97, 0:1008])
            osrc = bass.AP(ob0[:].tensor, 0,
                           [[32 * OBW, 3], [MS, 16], [1, MO]])
            nc.gpsimd.dma_start(out[0, 0, 0:48].unsqueeze(0), osrc)
            osrc = bass.AP(ob0[:].tensor, 96 * OBW,
                           [[OBW, 1], [MS, 8], [1, MO]])
            nc.gpsimd.dma_start(out[0, 0, 48:MO].unsqueeze(0), osrc)

            # ---- s1 scratch + stage-2: the late tile's chain is the
            # tail; everything else precedes it in the ring
            nc.sync.dma_start(scs[1][:, 0:EW], ple1[:])
            nc.sync.dma_start(scs[1][:, EW:F], pll1[:])
            aA_1 = gather_dma(1, 0, 3136, "aA")
            alate_1 = gather_dma(1, 3136, 385, "alate")
            ps2_1 = psum2.tile([128, 2 * BLK], f32, tag="ps2", name="ps2_1")
            ps3 = psum3.tile([128, BLK], f32, tag="ps3", name="ps3_1")
            mm2(aA_1, 0, 0, 3024, ps2_1)
            mm2(aA_1, 0, 3024, 3136, ps3)
            mm2(alate_1, 3136, 3136, W, ps3)
            ob3 = opool.tile([128, BLK], f32, tag="ob3", name="obuf3_1")
            nc.scalar.copy(ob3[96:97, 0:504], ps3[96:97, 0:504])
            osrc = bass.AP(ob3[:].tensor, 96 * BLK,
                           [[BLK, 1], [MS, 8], [1, MO]])
            nc.sync.dma_start(out[1, 0, 48:MO].unsqueeze(0), osrc)
            ob1 = opool.tile([128, OBW], f32, tag="ob1", name="obuf1")
            nc.vector.tensor_copy(ob1[0:65, 0:1008], ps2_1[0:65, 0:1008])
            osrc = bass.AP(ob1[:].tensor, 0,
                           [[32 * OBW, 3], [MS, 16], [1, MO]])
            nc.sync.dma_start(out[1, 0, 0:48].unsqueeze(0), osrc)

    nc.compile()
    return nc


def _get_nc():
    if "nc" not in _CACHE:
        _CACHE["nc"] = _build()
    return _CACHE["nc"]


def _run(z, x, weights, **runkw):
    z = np.ascontiguousarray(np.asarray(z), dtype=np.float32)
    x = np.ascontiguousarray(np.asarray(x), dtype=np.float32)
    w = np.asarray(weights, dtype=np.float32).reshape(C)
    # host re-layout (no arithmetic): channel k*128+c -> partition c,
    # chunk k, so the tiny loads are contiguous per partition
    zl = np.ascontiguousarray(
        z.reshape(N, 2, 128, KK).transpose(0, 2, 1, 3)
    )
    wl = np.ascontiguousarray(
        np.broadcast_to(
            w.reshape(2, 128).T[:, :, None], (128, 2, KK)
        )
    )
    in_maps = []
    for i in range(NCORES):
        lo, hi = i * SPC, (i + 1) * SPC
        in_maps.append({"z": zl[lo:hi], "x": x[lo:hi], "w": wl})
    nc = _get_nc()
    try:
        res = run_bass_kernel_spmd(
            nc, in_maps, core_ids=list(range(NCORES)), **runkw
        )
    except Exception:
        res = run_bass_kernel_spmd(
            nc, in_maps, core_ids=list(range(NCORES)), **runkw
        )
    full = np.concatenate([res.results[i]["out"] for i in range(NCORES)], axis=0)
    return full, res


def kernel(z, x, weights):
    full, _ = _run(z, x, weights)
    return full
